# revision 23
# baseline (speedup 1.0000x reference)
"""MinGRU Trainium2 kernel.

Problem: B=8, T=4096, D=512, H=512 MinGRU:
    k = x @ Wz^T + bz;  z = sigmoid(k)
    w = x @ Wh^T + bh;  h~ = g(w),  g(w) = relu(w) + 0.5 (w>=0) | sigmoid(w) (w<0)
    h_t = (1 - z_t) * h_{t-1} + z_t * h~_t,   h_{-1} = g(h_0)
(The reference computes this recurrence in log space via cumlogsumexp; in
linear space all quantities are positive and bounded, so a direct scan with
fp32 state is numerically stable.)

Sharding: data-parallel over batch, one batch row per NeuronCore (8 cores).

Per-core device layout (everything transposed so H sits on partitions and T
on the free dim, which lets the VectorE `tensor_tensor_scan` instruction run
the recurrence along T):
    x8  (D=512, T)  fp8 e4m3 (x * 8)  - z-path GEMM rhs, DoubleRow mode
    xb  (D=512, T-1024) bf16          - h-path GEMM rhs for chunks >= 2
    wz8/wh8 (D, H) fp8 (W^T * 32), whb (D, H) bf16 - stationary weights
    kp = x8 @ wz8 accumulated fp8 DoubleRow (2 k-pairs of 256)  [PE]
    wp = x8 @ wh8 (chunks 0-1) or xb @ whb (chunks 2+)          [PE]
    a    = sigmoid(-kp/256 - bz)                 [ScalarE, scale+bias fused]
    z    = sigmoid(kp/256 + bz)                  [ScalarE]
    s    = sigmoid(wp*hs + bh)                   [ScalarE]
    v    = wp*hs + bh + 0.5   (Identity act)     [ScalarE]
    g    = max(s, v)                             [VectorE tensor_tensor, 2x]
           (identity: g(w) = max(sigmoid(w), w + 0.5), exact)
    bn   = z * g                                 [VectorE tensor_tensor, 2x]
    h    = scan: state = a*state + bn            [VectorE tensor_tensor_scan,
                                                  fp32 internal state; carry
                                                  chained via prev h[:, -1:]]
    hT out (H, T) bf16 -> host transposes back

v2 changes vs the 97us baseline (measured bottleneck: VectorE ~96% busy in
steady state; exec = head(20us) + V-work(73us) + tail(5us)):
  - g via max(sigmoid, w+0.5): 1 DVE tensor_tensor instead of min+add
    (saves ~0.2ns/col of DVE + instr overhead); the affine lives on ScalarE
    as an Identity activation (Scalar had slack).
  - scan carry chained directly to the previous h tile's last column
    (drops 20 CAST copies from DVE).
  - head: first two chunks run the h-path GEMM from the fp8 x against an
    fp8 Wh copy (DoubleRow), so compute needs only wz8+wh8+x8[0:1024]
    (768KB) instead of +whb+xb (1.5MB+). Inputs split across BOTH HWDGE
    rings (sync + scalar) to parallelize the head load.
  - z stays on ScalarE everywhere (the old ci==2 DVE-z balancing moved work
    to the wrong engine once the DVE diet landed).

Things measured NOT to work here: GpSimd elementwise (11.8us per
128x1024 tile, plus SBUF-port contention slowing DVE ~3.5x), DVE
scalar_tensor_tensor (no packed mode, ~2 cyc/elem), SWDGE bulk DMA
(~4x slower per descriptor than HWDGE), pair/tree-compressing the scan
(t_t at 2x costs ~as much as the scan cols saved, plus overheads).
"""

import os

import numpy as np

import concourse.bass as bass
import concourse.mybir as mybir
import concourse.tile as tile
from concourse import bacc
from concourse.bass_utils import run_bass_kernel_spmd

# Problem constants (hardcoded per harness contract).
B, T, D, H = 8, 4096, 512, 512
P = 128          # partitions
DB = D // P      # 4 contraction blocks
HB = H // P      # 4 output h blocks
MM_N = 512       # matmul free-dim chunk (one PSUM bank)
XS = 8.0         # fp8 input scale
WS = 32.0        # fp8 weight scale

F32 = mybir.dt.float32
BF16 = mybir.dt.bfloat16
F8 = mybir.dt.float8e4
EW = BF16        # elementwise chain dtype

# chunks 0-1 run the h-path GEMM fully in fp8 (x8 @ wh8); xb only ships
# columns XB0..T. (2048-col V-spans and in-place tensor_tensor were both
# measured SLOWER: wide DVE ops lose ~30% throughput and the big scans
# drain the pipeline.)
CHUNKS = [256, 768, 1024, 1024, 768, 256]
FP8H = {0, 1, 2, 3}
DVEZ = {2, 3}
XB0 = 3072
assert sum(CHUNKS) == T

# Stash of the last run's BassKernelResults (for test harness introspection).
LAST_RESULT = None


def _build_nc():
    nc = bacc.Bacc(
        "TRN2",
        target_bir_lowering=False,
        debug=False,
        enable_asserts=False,
        num_devices=B,
    )

    # Partition-major layouts: one dma_start moves a whole tensor region
    # (the source AP walks (p, db, t) exactly like the SBUF destination), so
    # descriptor generation (~0.6us per dma_start) stays off the hot path.
    x8_d = nc.dram_tensor("x8", (P, DB, T), F8, kind="ExternalInput")
    xb_d = nc.dram_tensor("xb", (P, DB, T - XB0), BF16, kind="ExternalInput")
    wz8_d = nc.dram_tensor("wz8", (P, DB, H), F8, kind="ExternalInput")
    wh8_d = nc.dram_tensor("wh8", (P, DB, H), F8, kind="ExternalInput")
    whb_d = nc.dram_tensor("whb", (P, DB, H), BF16, kind="ExternalInput")
    # smalls columns: [0:4] -bz per h-block, [4:8] bh, [8:12] g(h_0)
    # carries, [12:16] +bz, [16:20] bh+0.5
    smalls_d = nc.dram_tensor("smalls", (P, 24), F32, kind="ExternalInput")
    hT_d = nc.dram_tensor("hT", (H, T), EW, kind="ExternalOutput")

    AF = mybir.ActivationFunctionType
    OP = mybir.AluOpType
    DR = mybir.MatmulPerfMode.DoubleRow

    from contextlib import ExitStack

    with tile.TileContext(nc) as tc, ExitStack() as ctx:
        wpool = ctx.enter_context(tc.tile_pool(name="weights", bufs=1))
        xpool = ctx.enter_context(tc.tile_pool(name="xres", bufs=1))
        spool = ctx.enter_context(tc.tile_pool(name="work", bufs=5))
        hpool = ctx.enter_context(tc.tile_pool(name="hout", bufs=6))
        ppool = ctx.enter_context(tc.tile_pool(name="psum", bufs=2, space="PSUM"))

        smalls = wpool.tile([P, 24], F32, name="smalls")
        wz8_sb = wpool.tile([P, DB, H], F8, name="wz8_sb")
        wh8_sb = wpool.tile([P, DB, H], F8, name="wh8_sb")
        whb_sb = wpool.tile([P, DB, H], BF16, name="whb_sb")
        x8_sb = xpool.tile([P, DB, T], F8, name="x8_sb")
        xb_sb = xpool.tile([P, DB, T - XB0], BF16, name="xb_sb")

        # Warmup/dummy tiles first: their memsets are instant and unblock
        # the dummy activations + PE warmup below.
        dwa = wpool.tile([P, 128], BF16, name="dwa")
        nc.vector.memset(dwa[:], 0.0)
        dwb = wpool.tile([P, 256], BF16, name="dwb")
        nc.vector.memset(dwb[:], 0.0)
        # Force the ACT table-set load at t~0, while the DMA queues are
        # still empty (sigmoid/identity/relu share one table set).
        dact = wpool.tile([P, 16], EW, name="dact")
        nc.scalar.activation(dact[:], dwa[:, :16], AF.Sigmoid)
        nc.scalar.activation(dact[:], dwa[:, :16], AF.Relu)

        # Input DMAs: all on the sync HWDGE ring, in global need order. The
        # two rings (sync + scalar) share the underlying DMA engines, so
        # transfers serialize globally in ISSUE order regardless of ring —
        # a second ring buys no bandwidth, risks scheduler reorders, blocks
        # ScalarE at sem-reuse stalls, and slows the exit drain. Warm DMA
        # runs ~400GB/s, so the head-critical 1MB (wz8+x8[0:512]+wh8) lands
        # by ~11us and chunks 2+ have multi-us margins.
        nc.sync.dma_start(smalls[:], smalls_d.ap()[:])
        nc.sync.dma_start(wz8_sb[:], wz8_d.ap()[:])
        nc.sync.dma_start(x8_sb[:, :, :512], x8_d.ap()[:, :, :512])
        nc.sync.dma_start(wh8_sb[:], wh8_d.ap()[:])
        nc.sync.dma_start(x8_sb[:, :, 512:1024], x8_d.ap()[:, :, 512:1024])
        nc.sync.dma_start(x8_sb[:, :, 1024:2048], x8_d.ap()[:, :, 1024:2048])
        nc.sync.dma_start(x8_sb[:, :, 2048:], x8_d.ap()[:, :, 2048:])
        nc.sync.dma_start(whb_sb[:], whb_d.ap()[:])
        nc.sync.dma_start(xb_sb[:], xb_d.ap()[:])

        # PE p-state warmup: stream dummy matmuls while the setup DMAs are
        # in flight so the clock is ramping when real matmuls start (cold PE
        # runs at ~1/3 speed; keep the dummies small so they finish fast).
        for _ in range(10):
            dp = ppool.tile([P, 256], F32, name="dp", tag="kp")
            nc.tensor.matmul(dp[:], dwa[:], dwb[:], start=True, stop=True)

        # --- Main loops: T chunks outer (the 4 h-blocks' scan chains stay
        # independent, so consecutive DVE scans never wait on each other),
        # h-block inner. Small first chunks prime the pipeline; small last
        # chunk shortens the serial tail. ---
        starts = [sum(CHUNKS[:i]) for i in range(len(CHUNKS))]

        prev_h = [None] * HB
        deferred = []
        for ci, (ts0, clen) in enumerate(zip(starts, CHUNKS)):
            fp8h = ci in FP8H
            # Scalar/DVE balance: on the DVEZ chunks the DVE derives t = a-1
            # (= -z) with a 4x-packed tensor_scalar and the scan runs
            # op1=subtract, dropping the z activation from ScalarE.
            use_z = ci not in DVEZ
            for hb in range(HB):
                hs = slice(hb * P, (hb + 1) * P)

                # z-path GEMM: fp8 DoubleRow, 2 k-pairs of 256 contraction
                kp = ppool.tile([P, 1024], F32, name="kp", tag="kp")
                for cc in range(0, clen, MM_N):
                    mmn = min(MM_N, clen - cc)
                    cs = slice(ts0 + cc, ts0 + cc + mmn)
                    for p2 in (0, 2):
                        nc.tensor.matmul(
                            kp[:, cc:cc + mmn],
                            wz8_sb[:, p2:p2 + 2, hs],
                            x8_sb[:, p2:p2 + 2, cs],
                            start=(p2 == 0), stop=(p2 == 2),
                            perf_mode=DR,
                        )

                a_t = spool.tile([P, 1024], EW, name="a_t", tag="a")
                nc.scalar.activation(
                    a_t[:, :clen], kp[:, :clen], AF.Sigmoid,
                    bias=smalls[:, hb:hb + 1], scale=-1.0 / (XS * WS),
                )
                if use_z:
                    z_t = spool.tile([P, 1024], EW, name="z_t", tag="z")
                    nc.scalar.activation(
                        z_t[:, :clen], kp[:, :clen], AF.Sigmoid,
                        bias=smalls[:, 12 + hb:13 + hb], scale=1.0 / (XS * WS),
                    )

                # h-path GEMM: fp8 DoubleRow for the head chunks, bf16 after
                wp = ppool.tile([P, 1024], F32, name="wp", tag="wp")
                for cc in range(0, clen, MM_N):
                    mmn = min(MM_N, clen - cc)
                    if fp8h:
                        cs = slice(ts0 + cc, ts0 + cc + mmn)
                        for p2 in (0, 2):
                            nc.tensor.matmul(
                                wp[:, cc:cc + mmn],
                                wh8_sb[:, p2:p2 + 2, hs],
                                x8_sb[:, p2:p2 + 2, cs],
                                start=(p2 == 0), stop=(p2 == 2),
                                perf_mode=DR,
                            )
                    else:
                        cs = slice(ts0 - XB0 + cc, ts0 - XB0 + cc + mmn)
                        for db in range(DB):
                            nc.tensor.matmul(
                                wp[:, cc:cc + mmn],
                                whb_sb[:, db, hs],
                                xb_sb[:, db, cs],
                                start=(db == 0), stop=(db == DB - 1),
                            )

                s_t = spool.tile([P, 1024], EW, name="s_t", tag="s")
                v_t = spool.tile([P, 1024], EW, name="v_t", tag="v")
                wscale = 1.0 / (XS * WS) if fp8h else 1.0
                nc.scalar.activation(
                    s_t[:, :clen], wp[:, :clen], AF.Sigmoid,
                    bias=smalls[:, 4 + hb:5 + hb], scale=wscale,
                )
                # Relu here is exact: when relu clips (w+bh+0.5 < 0), the max
                # below picks sigmoid anyway (s > 0 always). Relu shares the
                # sigmoid act-table set, avoiding a second ACT_TABLE_LOAD.
                nc.scalar.activation(
                    v_t[:, :clen], wp[:, :clen], AF.Relu,
                    bias=smalls[:, 16 + hb:17 + hb], scale=wscale,
                )

                # g = max(s, v) -- exact: g(w) = max(sigmoid(w), w + 0.5).
                # Then bn = z * g (or t*g = -z*g with op1=subtract on the
                # DVE-z chunks). tensor_tensor at bf16 2x.
                g_t = spool.tile([P, 1024], EW, name="g_t", tag="g")
                nc.vector.tensor_max(g_t[:, :clen], s_t[:, :clen], v_t[:, :clen])
                bn_t = spool.tile([P, 1024], EW, name="bn_t", tag="bn")
                if use_z:
                    nc.vector.tensor_mul(bn_t[:, :clen], z_t[:, :clen], g_t[:, :clen])
                else:
                    t_t = spool.tile([P, 1024], EW, name="t_t", tag="z")
                    nc.vector.tensor_scalar_sub(t_t[:, :clen], a_t[:, :clen], 1.0)
                    nc.vector.tensor_mul(bn_t[:, :clen], t_t[:, :clen], g_t[:, :clen])

                op1 = OP.add if use_z else OP.subtract
                init = (smalls[:, 8 + hb:9 + hb] if ci == 0
                        else prev_h[hb][0][:, prev_h[hb][1] - 1:prev_h[hb][1]])
                last_chunk = ci == len(CHUNKS) - 1
                if last_chunk:
                    # Defer the scans: queue all four h-blocks' prep work
                    # first so the tail is just 4 short scans + DMAs.
                    deferred.append((hb, hs, a_t, bn_t, init, op1))
                    continue
                h_t = hpool.tile([P, 1024], EW, name="h_t", tag="h")
                nc.vector.tensor_tensor_scan(
                    h_t[:, :clen], a_t[:, :clen], bn_t[:, :clen],
                    init, op0=OP.mult, op1=op1,
                )
                nc.sync.dma_start(
                    hT_d.ap()[hs, ts0:ts0 + clen], h_t[:, :clen]
                )
                prev_h[hb] = (h_t, clen)

        # Tail: the last chunk's four scans, one whole scan + one DMA per
        # h-block (splitting into 128-col pieces made 256B-row DMAs whose
        # descriptor-bound flush stalled the exit drain by ~5us).
        ts0, clen = starts[-1], CHUNKS[-1]
        for hb, hs, a_t, bn_t, init, op1 in deferred:
            h_t = hpool.tile([P, 1024], EW, name="h_t", tag="h")
            nc.vector.tensor_tensor_scan(
                h_t[:, :clen], a_t[:, :clen], bn_t[:, :clen], init,
                op0=OP.mult, op1=op1,
            )
            nc.sync.dma_start(
                hT_d.ap()[hs, ts0:ts0 + clen], h_t[:, :clen]
            )

    nc.compile()
    return nc


def _host_prep(x, h_0, Wz, bz, Wh, bh):
    x = np.asarray(x, dtype=np.float32)
    h_0 = np.asarray(h_0, dtype=np.float32)
    Wz = np.asarray(Wz, dtype=np.float32)
    bz = np.asarray(bz, dtype=np.float32)
    Wh = np.asarray(Wh, dtype=np.float32)
    bh = np.asarray(bh, dtype=np.float32)

    import ml_dtypes
    bf16 = ml_dtypes.bfloat16
    f8 = ml_dtypes.float8_e4m3
    xT = np.transpose(x, (0, 2, 1))                      # (B, D, T)
    # (B, P, DB, T): partition-major so device DMAs batch per tensor.
    xb = np.ascontiguousarray(
        xT[:, :, XB0:].astype(bf16).reshape(B, DB, P, T - XB0)
        .transpose(0, 2, 1, 3))
    x8 = np.ascontiguousarray(
        (xT * XS).astype(f8).reshape(B, DB, P, T).transpose(0, 2, 1, 3))
    wz8 = np.ascontiguousarray(
        (Wz.T * WS).astype(f8).reshape(DB, P, H).transpose(1, 0, 2))
    wh8 = np.ascontiguousarray(
        (Wh.T * WS).astype(f8).reshape(DB, P, H).transpose(1, 0, 2))
    whb = np.ascontiguousarray(
        Wh.T.astype(bf16).reshape(DB, P, H).transpose(1, 0, 2))

    # initial carry: g(h_0) = min(sigmoid(h_0), 0.5) + relu(h_0)
    sig = 1.0 / (1.0 + np.exp(-h_0.astype(np.float64)))
    h0g = (np.minimum(sig, 0.5) + np.maximum(h_0, 0.0)).astype(np.float32)

    smalls = np.zeros((B, P, 24), dtype=np.float32)
    for hb in range(HB):
        blk = slice(hb * P, (hb + 1) * P)
        smalls[:, :, hb] = -bz[blk]
        smalls[:, :, 4 + hb] = bh[blk]
        smalls[:, :, 8 + hb] = h0g[:, blk]
        smalls[:, :, 12 + hb] = bz[blk]
        smalls[:, :, 16 + hb] = bh[blk] + 0.5
    smalls = np.ascontiguousarray(smalls)

    in_maps = []
    for i in range(B):
        in_maps.append({
            "x8": x8[i],
            "xb": xb[i],
            "wz8": wz8,
            "wh8": wh8,
            "whb": whb,
            "smalls": smalls[i],
        })
    return in_maps


def kernel(x, h_0, Wz, bz, Wh, bh):
    global LAST_RESULT
    in_maps = _host_prep(x, h_0, Wz, bz, Wh, bh)
    nc = _build_nc()
    res = run_bass_kernel_spmd(
        nc,
        in_maps,
        core_ids=list(range(B)),
        trace=bool(int(os.environ.get("MINGRU_TRACE", "0"))),
    )
    LAST_RESULT = res
    out = np.empty((B, T, H), dtype=np.float32)
    for i in range(B):
        out[i] = np.asarray(res.results[i]["hT"]).astype(np.float32).T
    return out
